# revision 10
# baseline (speedup 1.0000x reference)
"""Bass/Trainium2 kernel for nn_MultiHeadAttention_85615878078828.

Full (unsharded) inputs in, full output out. Sharding: 8 cores =
2 batches x 4 head-groups (tensor-parallel on heads + data-parallel on
batch). Each core runs QKV projection + causal attention for its 4
heads + a partial output projection; the host sums the 4 partial
projections per batch element (the "all-reduce" of the TP out-proj,
done during unshard) and adds b_out.

Everything on-device is computed in transposed layout ([dim, seq]) so
no on-device transposes of activations are needed:
  - QKV_T  = W-stationary matmuls against x^T (host pre-transposes x)
  - scores = S_T[k, q] directly (2 heads packed on the PE array via
    row tiling, contract dim = 64 each)
  - causal mask folded into PSUM with an identity-matmul init
  - exp on ScalarE with fused 1/sqrt(D) scale, P_T stored bf16
  - P@V with ones-augmented V (M=65) -> softmax denominators fall out
    of the same accumulation (row 64)
  - normalization: reciprocal + gpsimd partition-broadcast + multiply
  - out-proj in transposed layout; host transposes back.
"""

import os
import sys
from contextlib import ExitStack

import numpy as np

sys.path.insert(0, "/opt/trn_rl_repo")

import ml_dtypes

BF16NP = ml_dtypes.bfloat16

import concourse.bass as bass
import concourse.tile as tile
from concourse import bacc, mybir
from concourse.bass_utils import run_bass_kernel_spmd

# Problem shapes (hardcoded per contract)
B, S, H, N = 2, 2048, 1024, 16
D = H // N  # 64
P = 128
SCALE = float(D) ** -0.5  # 0.125
NEG = -1.0e30

F32 = mybir.dt.float32
F32R = mybir.dt.float32r
BF16 = mybir.dt.bfloat16

# P_T triangular-ish storage offsets: chunk kc stores global cols
# [512*(kc//4), 2048) at offset OFF[kc]
OFF = [0] * 17
for _kc in range(16):
    OFF[_kc + 1] = OFF[_kc] + (S - 512 * (_kc // 4))
PT_TOT = OFF[16]  # 20480
DEBUG = False


def _emit(nc, tc, ctx):
    add = mybir.AluOpType.add
    mult = mybir.AluOpType.mult
    Exp = mybir.ActivationFunctionType.Exp

    # ---- DRAM I/O ----
    xT = nc.dram_tensor("xT", [H, S], BF16, kind="ExternalInput").ap()
    wqkT = nc.dram_tensor("wqkT", [H, 512], BF16, kind="ExternalInput").ap()
    wvT = nc.dram_tensor("wvT", [H, 256], BF16, kind="ExternalInput").ap()
    bqkv = nc.dram_tensor("bqkv", [768], F32, kind="ExternalInput").ap()
    woT = nc.dram_tensor("woT", [256, H], BF16, kind="ExternalInput").ap()
    maskd = nc.dram_tensor("maskd", [P, P], BF16, kind="ExternalInput").ap()
    identd = nc.dram_tensor("identd", [P, P], BF16, kind="ExternalInput").ap()
    outT = nc.dram_tensor("outT", [H, S], F32, kind="ExternalOutput").ap()
    dbg = {}
    if DEBUG:
        dbg["qt0"] = nc.dram_tensor("dbg_qt0", [P, S], BF16, kind="ExternalOutput").ap()
        dbg["kt0"] = nc.dram_tensor("dbg_kt0", [P, S], BF16, kind="ExternalOutput").ap()
        dbg["vaug0"] = nc.dram_tensor("dbg_vaug0", [P, 16, 65], BF16, kind="ExternalOutput").ap()
        dbg["attn0"] = nc.dram_tensor("dbg_attn0", [P, S], BF16, kind="ExternalOutput").ap()
        dbg["pt00"] = nc.dram_tensor("dbg_pt00", [P, 2048], BF16, kind="ExternalOutput").ap()
        dbg["sums"] = nc.dram_tensor("dbg_sums", [1, 512], F32, kind="ExternalOutput").ap()
        dbg["rec"] = nc.dram_tensor("dbg_rec", [1, 512], F32, kind="ExternalOutput").ap()
        dbg["rbc"] = nc.dram_tensor("dbg_rbc", [64, 512], F32, kind="ExternalOutput").ap()

    # ---- persistent pools ----
    const = ctx.enter_context(tc.tile_pool(name="const", bufs=1))
    qkp = ctx.enter_context(tc.tile_pool(name="qk", bufs=1))
    vaugp = ctx.enter_context(tc.tile_pool(name="vaug", bufs=1))
    attnp = ctx.enter_context(tc.tile_pool(name="attn", bufs=1))
    smallp = ctx.enter_context(tc.tile_pool(name="small", bufs=3))
    oevacp = ctx.enter_context(tc.tile_pool(name="oevac", bufs=3))

    wo_sb = const.tile([P, 2, H], BF16, tag="wo")
    nc.sync.dma_start(wo_sb[:], woT.rearrange("(c p) j -> p c j", p=P))
    mask_sb = const.tile([P, P], BF16, tag="mask")
    nc.sync.dma_start(mask_sb[:], maskd)
    id_sb = const.tile([P, P], BF16, tag="ident")
    nc.sync.dma_start(id_sb[:], identd)

    # Q_T / K_T per head-pair: [128 (2x64 dims), 2048]
    qt = [qkp.tile([P, S], BF16, tag=f"qt{i}", name=f"qt{i}") for i in range(2)]
    kt = [qkp.tile([P, S], BF16, tag=f"kt{i}", name=f"kt{i}") for i in range(2)]
    # V augmented with a ones column: per head [128 k, 16 kc, 65]
    vaug = [vaugp.tile([P, 16, 65], BF16, tag=f"vaug{h}", name=f"vaug{h}") for h in range(4)]
    for h in range(4):
        nc.gpsimd.memset(vaug[h][:, :, 64:65], 1.0)
    # attention output (transposed): per pair [128 ch, 2048 s]
    attn = [attnp.tile([P, S], BF16, tag=f"attn{i}", name=f"attn{i}") for i in range(2)]

    # ---- phase 1: QKV projection (transposed layout) ----
    with (
        tc.tile_pool(name="xp", bufs=1) as xp,
        tc.tile_pool(name="vtp", bufs=1) as vtp,
        tc.tile_pool(name="ps1", bufs=4, space="PSUM") as ps1,
    ):
        wqk_sb = xp.tile([P, 8, 512], BF16, tag="wqk")
        nc.sync.dma_start(wqk_sb[:], wqkT.rearrange("(o p) c -> p o c", p=P))
        wv_sb = xp.tile([P, 8, 256], BF16, tag="wv")
        nc.sync.dma_start(wv_sb[:], wvT.rearrange("(o p) c -> p o c", p=P))
        b_sb = xp.tile([P, 6], F32, tag="b")
        nc.sync.dma_start(b_sb[:], bqkv.rearrange("(c p) -> p c", p=P))
        x_sb = xp.tile([P, 8, S], BF16, tag="x")
        x_r = xT.rearrange("(o p) s -> p o s", p=P)
        for o in range(8):
            nc.sync.dma_start(x_sb[:, o, :], x_r[:, o, :])
        vt = [vtp.tile([P, S], BF16, tag=f"vt{i}", name=f"vt{i}") for i in range(2)]

        for pc in range(6):
            if pc < 4:
                w_ch = wqk_sb[:, :, 128 * pc : 128 * (pc + 1)]
                dst = (qt + kt)[pc]
            else:
                w_ch = wv_sb[:, :, 128 * (pc - 4) : 128 * (pc - 3)]
                dst = vt[pc - 4]
            for sb in range(4):
                ps = ps1.tile([P, 512], F32, tag="ps1", name="ps1")
                for o in range(8):
                    nc.tensor.matmul(
                        ps[:],
                        w_ch[:, o, :],
                        x_sb[:, o, 512 * sb : 512 * (sb + 1)],
                        start=(o == 0),
                        stop=(o == 7),
                    )
                # evacuate with bias add (bias per output-channel = partition)
                nc.vector.tensor_scalar(
                    out=dst[:, 512 * sb : 512 * (sb + 1)],
                    in0=ps[:],
                    scalar1=b_sb[:, pc : pc + 1],
                    scalar2=None,
                    op0=add,
                )

        # V_T -> V via DMA xbar transpose (bf16) through a DRAM bounce.
        # The xbar-transpose destination MUST be contiguous (non-contiguous
        # dst produces wrong output on HW), so transpose into vkd and
        # engine-copy into the ones-augmented tiles.
        vdram = [
            nc.dram_tensor(f"vdram{i}", [P, S], BF16).ap() for i in range(2)
        ]
        for pair in range(2):
            nc.sync.dma_start(vdram[pair], vt[pair][:])
        for h in range(4):
            pair, sub = divmod(h, 2)
            vkd = vtp.tile([P, 16, 64], BF16, tag=f"vkd{h}", name=f"vkd{h}")
            nc.sync.dma_start_transpose(
                vkd[:], vdram[pair][64 * sub : 64 * sub + 64, :]
            )
            nc.vector.tensor_copy(vaug[h][:, :, 0:64], vkd[:])

    # ---- phase 2: attention per head-pair ----
    with (
        tc.tile_pool(name="pp", bufs=1) as pp,
        tc.tile_pool(name="scps", bufs=3, space="PSUM") as scps,
        tc.tile_pool(name="pvps", bufs=2, space="PSUM") as pvps,
    ):
        for pair in range(2):
            pt = [pp.tile([P, PT_TOT], BF16, tag=f"pt{s}", name=f"pt{s}") for s in range(2)]
            if DEBUG and pair == 1:
                pass
            for j in range(4):
                for kc in range(4 * j, 4 * j + 4):
                    d = kc - 4 * j
                    g0 = (128 * kc) // 1024
                    st = {}
                    for sub in range(2):
                        for g in range(g0, 2):
                            st[sub, g] = scps.tile([P, 1024], F32, tag="st", name="st")
                    # causal-mask init of the diagonal 128x128 block
                    # (start=True clears the whole containing bank)
                    lc = 128 * kc - 1024 * g0
                    for sub in range(2):
                        nc.tensor.matmul(
                            st[sub, g0][:, lc : lc + 128],
                            id_sb,
                            mask_sb,
                            start=True,
                            stop=False,
                        )
                    # scores S_T[k, q], both heads interleaved (row packing).
                    # The diagonal block is split at the 128-col boundary:
                    # first 128 cols accumulate onto the mask init, the
                    # remainder of the bank overwrites (has_written clear).
                    for jb in range(j, 4):
                        segs = []
                        if jb == j:
                            segs.append((128 * kc, 128, False, d == 3))
                            if d < 3:
                                segs.append(
                                    (128 * kc + 128, 512 * (j + 1) - 128 * kc - 128,
                                     False, True)
                                )
                        else:
                            segs.append((512 * jb, 512, True, True))
                        for n0, ln, sflag, eflag in segs:
                            g = n0 // 1024
                            l0 = n0 - 1024 * g
                            for sub in range(2):
                                o0 = 64 * sub
                                nc.tensor.matmul(
                                    st[sub, g][:, l0 : l0 + ln],
                                    kt[pair][o0 : o0 + 64, 128 * kc : 128 * kc + 128],
                                    qt[pair][o0 : o0 + 64, n0 : n0 + ln],
                                    start=sflag,
                                    stop=eflag,
                                )
                    # exp (scaled) PSUM -> P_T (bf16)
                    for sub in range(2):
                        for g in range(g0, 2):
                            l0 = max(0, 128 * kc - 1024 * g)
                            gl = 1024 - l0
                            q0c = 1024 * g + l0
                            po = OFF[kc] + (q0c - 512 * j)
                            nc.scalar.activation(
                                pt[sub][:, po : po + gl],
                                st[sub, g][:, l0 : l0 + gl],
                                Exp,
                                scale=SCALE,
                            )
                    # zero the below-diagonal strip [512j, 128kc)
                    if d > 0:
                        for sub in range(2):
                            nc.gpsimd.memset(
                                pt[sub][:, OFF[kc] : OFF[kc] + 128 * d], 0.0
                            )
                # P@V for query block j (plus denominator row 64)
                for sub in range(2):
                    h = 2 * pair + sub
                    pv = pvps.tile([P, 512], F32, tag="pv", name="pv")
                    for kc in range(4 * j + 4):
                        rl = OFF[kc] + 512 * j - 512 * (kc // 4)
                        nc.tensor.matmul(
                            pv[0:65, :],
                            vaug[h][:, kc, :],
                            pt[sub][:, rl : rl + 512],
                            start=(kc == 0),
                            stop=(kc == 4 * j + 3),
                        )
                    sums = smallp.tile([1, 512], F32, tag="sums", name="sums")
                    nc.vector.tensor_copy(sums[:], pv[64:65, :])
                    rec = smallp.tile([1, 512], F32, tag="rec", name="rec")
                    nc.vector.reciprocal_approx_fast(rec[:], sums[:])
                    rbc = smallp.tile([64, 512], F32, tag="rbc", name="rbc")
                    nc.gpsimd.partition_broadcast(rbc[:], rec[:])
                    nc.vector.tensor_tensor(
                        out=attn[pair][64 * sub : 64 * sub + 64, 512 * j : 512 * (j + 1)],
                        in0=pv[0:64, :],
                        in1=rbc[:],
                        op=mult,
                    )
                    if DEBUG and pair == 0 and j == 0 and sub == 0:
                        nc.sync.dma_start(dbg["sums"], sums[:])
                        nc.sync.dma_start(dbg["rec"], rec[:])
                        nc.sync.dma_start(dbg["rbc"], rbc[:])
                    if DEBUG and pair == 0 and j == 3 and sub == 1:
                        nc.sync.dma_start(dbg["pt00"], pt[0][:, 0:2048])

    if DEBUG:
        nc.sync.dma_start(dbg["qt0"], qt[0][:])
        nc.sync.dma_start(dbg["kt0"], kt[0][:])
        nc.sync.dma_start(dbg["vaug0"], vaug[0][:])
        nc.sync.dma_start(dbg["attn0"], attn[0][:])

    # ---- phase 3: output projection (transposed, partial) ----
    with tc.tile_pool(name="ops", bufs=2, space="PSUM") as ops:
        o_r = outT.rearrange("(o p) s -> p o s", p=P)
        for jc in range(8):
            for sb in range(4):
                ps = ops.tile([P, 512], F32, tag="ops", name="ops")
                for pc2 in range(2):
                    nc.tensor.matmul(
                        ps[:],
                        wo_sb[:, pc2, 128 * jc : 128 * (jc + 1)],
                        attn[pc2][:, 512 * sb : 512 * (sb + 1)],
                        start=(pc2 == 0),
                        stop=(pc2 == 1),
                    )
                ev = oevacp.tile([P, 512], F32, tag="ev", name="ev")
                nc.any.tensor_copy(out=ev[:], in_=ps[:])
                nc.sync.dma_start(o_r[:, jc, 512 * sb : 512 * (sb + 1)], ev[:])


_NC_CACHE = {}


def build_nc():
    if "nc" in _NC_CACHE:
        return _NC_CACHE["nc"]
    nc = bacc.Bacc(
        "TRN2",
        target_bir_lowering=False,
        debug=False,
        num_devices=8,
    )
    with tile.TileContext(nc) as tc:
        with ExitStack() as ctx:
            _emit(nc, tc, ctx)
    nc.compile()
    _NC_CACHE["nc"] = nc
    return nc


def make_in_maps(hidden_states, w_in, b_in, w_out):
    hidden_states = np.asarray(hidden_states, dtype=np.float32)
    w_in = np.asarray(w_in, dtype=np.float32)
    b_in = np.asarray(b_in, dtype=np.float32)
    w_out = np.asarray(w_out, dtype=np.float32)

    xT = [np.ascontiguousarray(hidden_states[b].T).astype(BF16NP) for b in range(B)]
    mask = np.where(
        np.arange(P)[:, None] <= np.arange(P)[None, :], 0.0, NEG
    ).astype(BF16NP)
    ident = np.eye(P, dtype=BF16NP)

    in_maps = []
    for c in range(8):
        b, hg = divmod(c, 4)
        q0 = 256 * hg
        wq = w_in[q0 : q0 + 256]
        wk = w_in[H + q0 : H + q0 + 256]
        wv = w_in[2 * H + q0 : 2 * H + q0 + 256]
        in_maps.append(
            {
                "xT": xT[b],
                "wqkT": np.ascontiguousarray(np.concatenate([wq, wk], 0).T).astype(BF16NP),
                "wvT": np.ascontiguousarray(wv.T).astype(BF16NP),
                "bqkv": np.ascontiguousarray(
                    np.concatenate(
                        [b_in[q0 : q0 + 256], b_in[H + q0 : H + q0 + 256],
                         b_in[2 * H + q0 : 2 * H + q0 + 256]]
                    )
                ),
                "woT": np.ascontiguousarray(w_out[:, q0 : q0 + 256].T).astype(BF16NP),
                "maskd": mask,
                "identd": ident,
            }
        )
    return in_maps


def _ensure_ntff_hook():
    """Provide antenv.axon_hooks (NTFF profiling hook) if the container's
    antenv stub lacks it, by driving the axon .so C ABI directly. Also
    neuter the S3 artifact upload (zero-egress container)."""
    import contextlib
    import ctypes
    import types

    import concourse.bass_utils as bu

    bu.upload_artifacts = lambda tmpdir: str(tmpdir)
    try:
        from antenv.axon_hooks import get_axon_ntff_profile_hook  # noqa: F401

        return
    except ImportError:
        pass
    import antenv

    so_path = "/opt/axon/libaxon_pjrt.so"
    hook = None
    try:
        lib = ctypes.CDLL(so_path)
        if hasattr(lib, "axon_start_nrt_profile"):
            lib.axon_start_nrt_profile.argtypes = [
                ctypes.POINTER(ctypes.c_int64),
                ctypes.c_size_t,
            ]
            lib.axon_start_nrt_profile.restype = ctypes.c_int64
            lib.axon_stop_nrt_profile.argtypes = [ctypes.c_char_p]
            lib.axon_stop_nrt_profile.restype = ctypes.c_int64

            @contextlib.contextmanager
            def _hook(output_dir, device_ids):
                import jax

                jax.devices()
                if device_ids:
                    ids = (ctypes.c_int64 * len(device_ids))(*device_ids)
                    rc = lib.axon_start_nrt_profile(ids, len(device_ids))
                else:
                    rc = lib.axon_start_nrt_profile(None, 0)
                if rc != 0:
                    raise RuntimeError(f"axon_start_nrt_profile rc={rc}")
                try:
                    yield
                finally:
                    n = lib.axon_stop_nrt_profile(str(output_dir).encode())
                    print(f"ntff profile: {n} file(s) -> {output_dir}")

            hook = _hook
    except OSError:
        hook = None

    mod = types.ModuleType("antenv.axon_hooks")
    mod.get_axon_ntff_profile_hook = lambda: hook
    mod.set_axon_ntff_profile_hook = lambda h: None
    sys.modules["antenv.axon_hooks"] = mod
    antenv.axon_hooks = mod


def run_device(hidden_states, w_in, b_in, w_out, b_out, trace=False):
    """Returns (full output, BassKernelResults)."""
    if trace:
        _ensure_ntff_hook()
    nc = build_nc()
    in_maps = make_in_maps(hidden_states, w_in, b_in, w_out)
    res = run_bass_kernel_spmd(
        nc, in_maps, core_ids=list(range(8)), trace=trace
    )
    out = np.zeros((B, S, H), dtype=np.float32)
    for c in range(8):
        out[c // 4] += res.results[c]["outT"].T
    out += np.asarray(b_out, dtype=np.float32)[None, None, :]
    return out, res


def kernel(hidden_states, w_in, b_in, w_out, b_out):
    out, _ = run_device(hidden_states, w_in, b_in, w_out, b_out, trace=False)
    return out


# revision 20
# speedup vs baseline: 1.2375x; 1.2375x over previous
"""Bass/Trainium2 kernel for nn_MultiHeadAttention_85615878078828.

Full (unsharded) inputs in, full output out. Sharding: 8 cores =
2 batches x 4 head-groups (tensor-parallel on heads + data-parallel on
batch). Each core runs QKV projection + causal attention for its 4
heads + a partial output projection; the host sums the 4 partial
projections per batch element (the "all-reduce" of the TP out-proj,
done during unshard) and adds b_out.

Everything on-device is computed in transposed layout ([dim, seq]) so
no on-device transposes of activations are needed:
  - QKV_T  = W-stationary matmuls against x^T (host pre-transposes x)
  - scores = S_T[k, q] directly (2 heads packed on the PE array via
    row tiling, contract dim = 64 each)
  - causal mask folded into PSUM with an identity-matmul init
  - exp on ScalarE with fused 1/sqrt(D) scale, P_T stored bf16
  - P@V with ones-augmented V (M=65) -> softmax denominators fall out
    of the same accumulation (row 64)
  - normalization: reciprocal + gpsimd partition-broadcast + multiply
  - out-proj in transposed layout; host transposes back.
"""

import os
import sys
from contextlib import ExitStack

import numpy as np

sys.path.insert(0, "/opt/trn_rl_repo")

import ml_dtypes

BF16NP = ml_dtypes.bfloat16

import concourse.bass as bass
import concourse.tile as tile
from concourse import bacc, mybir
from concourse.bass_utils import run_bass_kernel_spmd

# Problem shapes (hardcoded per contract)
B, S, H, N = 2, 2048, 1024, 16
D = H // N  # 64
P = 128
SCALE = float(D) ** -0.5  # 0.125
NEG = -1.0e30

F32 = mybir.dt.float32
F32R = mybir.dt.float32r
BF16 = mybir.dt.bfloat16

# P_T storage, split into two tiles per head by column half
# (g=0: query blocks 0-1, g=1: blocks 2-3) so pair-1's early exps only
# WAR against pair-0's early P@V reads. Chunk kc's g-half stores global
# cols [max(1024g, 512*(kc//4)), 1024*(g+1)) at offset OFFG[g][kc].
BS = [[], []]
WID = [[], []]
OFFG = [[0] * 17, [0] * 17]
for _kc in range(16):
    _bs0 = 512 * (_kc // 4)
    BS[0].append(_bs0)
    WID[0].append(max(0, 1024 - _bs0))
    _bs1 = max(1024, 512 * (_kc // 4))
    BS[1].append(_bs1)
    WID[1].append(2048 - _bs1)
    OFFG[0][_kc + 1] = OFFG[0][_kc] + WID[0][_kc]
    OFFG[1][_kc + 1] = OFFG[1][_kc] + WID[1][_kc]
PT_TOT0 = OFFG[0][16]  # 6144
PT_TOT1 = OFFG[1][16]  # 14336
DEBUG = False


def _emit(nc, tc, ctx):
    add = mybir.AluOpType.add
    mult = mybir.AluOpType.mult
    Exp = mybir.ActivationFunctionType.Exp

    # ---- DRAM I/O ----
    xT = nc.dram_tensor("xT", [H, S], BF16, kind="ExternalInput").ap()
    wqkT = nc.dram_tensor("wqkT", [H, 512], BF16, kind="ExternalInput").ap()
    wvT = nc.dram_tensor("wvT", [H, 256], BF16, kind="ExternalInput").ap()
    bqkv = nc.dram_tensor("bqkv", [768], F32, kind="ExternalInput").ap()
    woT = nc.dram_tensor("woT", [256, H], BF16, kind="ExternalInput").ap()
    maskd = nc.dram_tensor("maskd", [P, P], BF16, kind="ExternalInput").ap()
    identd = nc.dram_tensor("identd", [P, P], BF16, kind="ExternalInput").ap()
    outT = nc.dram_tensor("outT", [H, S], F32, kind="ExternalOutput").ap()
    dbg = {}
    if DEBUG:
        dbg["qt0"] = nc.dram_tensor("dbg_qt0", [P, S], BF16, kind="ExternalOutput").ap()
        dbg["kt0"] = nc.dram_tensor("dbg_kt0", [P, S], BF16, kind="ExternalOutput").ap()
        dbg["vaug0"] = nc.dram_tensor("dbg_vaug0", [P, 16, 65], BF16, kind="ExternalOutput").ap()
        dbg["attn0"] = nc.dram_tensor("dbg_attn0", [P, S], BF16, kind="ExternalOutput").ap()
        dbg["pt00"] = nc.dram_tensor("dbg_pt00", [P, 2048], BF16, kind="ExternalOutput").ap()
        dbg["sums"] = nc.dram_tensor("dbg_sums", [1, 512], F32, kind="ExternalOutput").ap()
        dbg["rec"] = nc.dram_tensor("dbg_rec", [1, 512], F32, kind="ExternalOutput").ap()
        dbg["rbc"] = nc.dram_tensor("dbg_rbc", [64, 512], F32, kind="ExternalOutput").ap()

    # ---- persistent pools ----
    const = ctx.enter_context(tc.tile_pool(name="const", bufs=1))
    qkp = ctx.enter_context(tc.tile_pool(name="qk", bufs=1))
    vaugp = ctx.enter_context(tc.tile_pool(name="vaug", bufs=1))
    attnp = ctx.enter_context(tc.tile_pool(name="attn", bufs=1))
    smallp = ctx.enter_context(tc.tile_pool(name="small", bufs=3))
    oevacp = ctx.enter_context(tc.tile_pool(name="oevac", bufs=3))


    # Q_T / K_T per head-pair: [128 (2x64 dims), 2048]
    qt = [qkp.tile([P, S], BF16, tag=f"qt{i}", name=f"qt{i}") for i in range(2)]
    kt = [qkp.tile([P, S], BF16, tag=f"kt{i}", name=f"kt{i}") for i in range(2)]
    # V augmented with a ones column: per head [128 k, 16 kc, 65]
    vaug = [vaugp.tile([P, 16, 65], BF16, tag=f"vaug{h}", name=f"vaug{h}") for h in range(4)]
    for h in range(4):
        nc.gpsimd.memset(vaug[h][:, :, 64:65], 1.0)
    # attention output (transposed): per pair [128 ch, 2048 s]
    attn = [attnp.tile([P, S], BF16, tag=f"attn{i}", name=f"attn{i}") for i in range(2)]

    # ---- phase 1: QKV projection (transposed layout) ----
    with (
        tc.tile_pool(name="xp", bufs=1) as xp,
        tc.tile_pool(name="vtp", bufs=1) as vtp,
        tc.tile_pool(name="ps1", bufs=4, space="PSUM") as ps1,
    ):
        wqk_sb = xp.tile([P, 8, 512], BF16, tag="wqk")
        nc.sync.dma_start(wqk_sb[:], wqkT.rearrange("(o p) c -> p o c", p=P))
        wv_sb = xp.tile([P, 8, 256], BF16, tag="wv")
        nc.sync.dma_start(wv_sb[:], wvT.rearrange("(o p) c -> p o c", p=P))
        b_sb = xp.tile([P, 6], F32, tag="b")
        nc.sync.dma_start(b_sb[:], bqkv.rearrange("(c p) -> p c", p=P))
        x_sb = xp.tile([P, 8, S], BF16, tag="x")
        x_r = xT.rearrange("(o p) s -> p o s", p=P)
        for o in range(8):
            nc.sync.dma_start(x_sb[:, o, :], x_r[:, o, :])
        vt = [vtp.tile([P, S], BF16, tag=f"vt{i}", name=f"vt{i}") for i in range(2)]

        for pc in range(6):
            if pc < 4:
                w_ch = wqk_sb[:, :, 128 * pc : 128 * (pc + 1)]
                dst = (qt + kt)[pc]
            else:
                w_ch = wv_sb[:, :, 128 * (pc - 4) : 128 * (pc - 3)]
                dst = vt[pc - 4]
            for sb in range(4):
                ps = ps1.tile([P, 512], F32, tag="ps1", name="ps1")
                for o in range(8):
                    nc.tensor.matmul(
                        ps[:],
                        w_ch[:, o, :],
                        x_sb[:, o, 512 * sb : 512 * (sb + 1)],
                        start=(o == 0),
                        stop=(o == 7),
                    )
                # evacuate with bias add (bias per output-channel = partition)
                nc.vector.tensor_scalar(
                    out=dst[:, 512 * sb : 512 * (sb + 1)],
                    in0=ps[:],
                    scalar1=b_sb[:, pc : pc + 1],
                    scalar2=None,
                    op0=add,
                )

        # V_T -> V via DMA xbar transpose (bf16) through a DRAM bounce.
        # The xbar-transpose destination MUST be contiguous (non-contiguous
        # dst produces wrong output on HW), so transpose into vkd and
        # engine-copy into the ones-augmented tiles.
        vdram = [
            nc.dram_tensor(f"vdram{i}", [P, S], BF16).ap() for i in range(2)
        ]
        for pair in range(2):
            nc.sync.dma_start(vdram[pair], vt[pair][:])
        for h in range(4):
            pair, sub = divmod(h, 2)
            vkd = vtp.tile([P, 16, 64], BF16, tag=f"vkd{h}", name=f"vkd{h}")
            nc.sync.dma_start_transpose(
                vkd[:], vdram[pair][64 * sub : 64 * sub + 64, :]
            )
            nc.vector.tensor_copy(vaug[h][:, :, 0:64], vkd[:])

    # ---- phase 2: attention per head-pair ----
    with (
        tc.tile_pool(name="pp", bufs=1) as pp,
        tc.tile_pool(name="scps", bufs=3, space="PSUM") as scps,
        tc.tile_pool(name="pvps", bufs=2, space="PSUM") as pvps,
    ):
        for pair in range(2):
            pt = [
            [pp.tile([P, PT_TOT0], BF16, tag=f"pt{s}g0", name=f"pt{s}g0"),
             pp.tile([P, PT_TOT1], BF16, tag=f"pt{s}g1", name=f"pt{s}g1")]
            for s in range(2)
        ]
            if DEBUG and pair == 1:
                pass
            for j in range(4):
                for kc in range(4 * j, 4 * j + 4):
                    d = kc - 4 * j
                    g0 = (128 * kc) // 1024
                    st = {}
                    for sub in range(2):
                        for g in range(g0, 2):
                            st[sub, g] = scps.tile([P, 1024], F32, tag="st", name="st")
                    # causal-mask init of the diagonal 128x128 block
                    # (start=True clears the whole containing bank)
                    lc = 128 * kc - 1024 * g0
                    for sub in range(2):
                        nc.tensor.matmul(
                            st[sub, g0][:, lc : lc + 128],
                            id_sb,
                            mask_sb,
                            start=True,
                            stop=False,
                        )
                    # scores S_T[k, q], both heads interleaved (row packing).
                    # The diagonal block is split at the 128-col boundary:
                    # first 128 cols accumulate onto the mask init, the
                    # remainder of the bank overwrites (has_written clear).
                    for jb in range(j, 4):
                        segs = []
                        if jb == j:
                            segs.append((128 * kc, 128, False, d == 3))
                            if d < 3:
                                segs.append(
                                    (128 * kc + 128, 512 * (j + 1) - 128 * kc - 128,
                                     False, True)
                                )
                        else:
                            segs.append((512 * jb, 512, True, True))
                        for n0, ln, sflag, eflag in segs:
                            g = n0 // 1024
                            l0 = n0 - 1024 * g
                            for sub in range(2):
                                o0 = 64 * sub
                                nc.tensor.matmul(
                                    st[sub, g][:, l0 : l0 + ln],
                                    kt[pair][o0 : o0 + 64, 128 * kc : 128 * kc + 128],
                                    qt[pair][o0 : o0 + 64, n0 : n0 + ln],
                                    start=sflag,
                                    stop=eflag,
                                )
                    # exp (scaled) PSUM -> P_T (bf16)
                    for sub in range(2):
                        for g in range(g0, 2):
                            l0 = max(0, 128 * kc - 1024 * g)
                            gl = 1024 - l0
                            q0c = 1024 * g + l0
                            po = OFF[kc] + (q0c - 512 * j)
                            nc.scalar.activation(
                                pt[sub][:, po : po + gl],
                                st[sub, g][:, l0 : l0 + gl],
                                Exp,
                                scale=SCALE,
                            )
                    # zero the below-diagonal strip [512j, 128kc)
                    if d > 0:
                        for sub in range(2):
                            nc.gpsimd.memset(
                                pt[sub][:, OFF[kc] : OFF[kc] + 128 * d], 0.0
                            )
                # P@V for query block j (plus denominator row 64)
                for sub in range(2):
                    h = 2 * pair + sub
                    pv = pvps.tile([P, 512], F32, tag="pv", name="pv")
                    gj = j // 2
                    for kc in range(4 * j + 4):
                        rl = OFFG[gj][kc] + 512 * j - BS[gj][kc]
                        nc.tensor.matmul(
                            pv[0:65, :],
                            vaug[h][:, kc, :],
                            pt[sub][gj][:, rl : rl + 512],
                            start=(kc == 0),
                            stop=(kc == 4 * j + 3),
                        )
                    sums = smallp.tile([1, 512], F32, tag="sums", name="sums")
                    nc.vector.tensor_copy(sums[:], pv[64:65, :])
                    rec = smallp.tile([1, 512], F32, tag="rec", name="rec")
                    nc.vector.reciprocal_approx_fast(rec[:], sums[:])
                    rbc = smallp.tile([64, 512], F32, tag="rbc", name="rbc")
                    nc.gpsimd.partition_broadcast(rbc[:], rec[:])
                    nc.vector.tensor_tensor(
                        out=attn[pair][64 * sub : 64 * sub + 64, 512 * j : 512 * (j + 1)],
                        in0=pv[0:64, :],
                        in1=rbc[:],
                        op=mult,
                    )
                    if DEBUG and pair == 0 and j == 0 and sub == 0:
                        nc.sync.dma_start(dbg["sums"], sums[:])
                        nc.sync.dma_start(dbg["rec"], rec[:])
                        nc.sync.dma_start(dbg["rbc"], rbc[:])
                    if DEBUG and pair == 0 and j == 3 and sub == 1:
                        nc.sync.dma_start(dbg["pt00"], pt[0][:, 0:2048])

    if DEBUG:
        nc.sync.dma_start(dbg["qt0"], qt[0][:])
        nc.sync.dma_start(dbg["kt0"], kt[0][:])
        nc.sync.dma_start(dbg["vaug0"], vaug[0][:])
        nc.sync.dma_start(dbg["attn0"], attn[0][:])

    # ---- phase 3: output projection (transposed, partial) ----
    with tc.tile_pool(name="ops", bufs=2, space="PSUM") as ops:
        o_r = outT.rearrange("(o p) s -> p o s", p=P)

    def outproj_group(jc, sb):
        """One (j-chunk, s-block) of the partial output projection."""
        ps = psml.tile([P, 512], F32, tag="psml", name="ops")
        for pc2 in range(2):
            nc.tensor.matmul(
                ps[:],
                wo_sb[:, pc2, 128 * jc : 128 * (jc + 1)],
                attn[pc2][:, 512 * sb : 512 * (sb + 1)],
                start=(pc2 == 0),
                stop=(pc2 == 1),
            )
        ev = oevacp.tile([P, 512], F32, tag="ev", name="ev")
        if jc % 2 == 0:
            nc.vector.tensor_copy(ev[:], ps[:])
        else:
            nc.scalar.copy(ev[:], ps[:])
        nc.sync.dma_start(o_r[:, jc, 512 * sb : 512 * (sb + 1)], ev[:])

    # pair 0: QKV up front, attention with pair-1 QKV as PE filler.
    # pair 1: attention with the out-projection as PE filler -- each
    # finished s-block's 8 projection groups are queued and drained
    # through the next block's score slots, so the in-order PE never
    # stalls behind a normalization chain.
    for ci in range(3):
        for sb in range(4):
            qkv_group(0, ci, sb)
    v_finish(0)
    attention(0, filler_items(1))

    import collections as _collections
    import itertools as _itertools

    oq = _collections.deque()
    attention(
        1,
        ((oq.popleft() if oq else None) for _ in _itertools.count()),
        per_j=lambda j: oq.extend(("op", jc, j) for jc in range(8)),
    )
    while oq:
        emit_filler(oq.popleft())


_NC_CACHE = {}


_NC_CACHE = {}


def build_nc():
    if "nc" in _NC_CACHE:
        return _NC_CACHE["nc"]
    nc = bacc.Bacc(
        "TRN2",
        target_bir_lowering=False,
        debug=False,
        num_devices=8,
    )
    with tile.TileContext(nc) as tc:
        with ExitStack() as ctx:
            _emit(nc, tc, ctx)
    nc.compile()
    _NC_CACHE["nc"] = nc
    return nc


def make_in_maps(hidden_states, w_in, b_in, w_out):
    hidden_states = np.asarray(hidden_states, dtype=np.float32)
    w_in = np.asarray(w_in, dtype=np.float32)
    b_in = np.asarray(b_in, dtype=np.float32)
    w_out = np.asarray(w_out, dtype=np.float32)

    xT = [np.ascontiguousarray(hidden_states[b].T).astype(BF16NP) for b in range(B)]
    mask = np.where(
        np.arange(P)[:, None] <= np.arange(P)[None, :], 0.0, NEG
    ).astype(BF16NP)
    ident = np.eye(P, dtype=BF16NP)

    in_maps = []
    for c in range(8):
        b, hg = divmod(c, 4)
        q0 = 256 * hg
        wq = w_in[q0 : q0 + 256]
        wk = w_in[H + q0 : H + q0 + 256]
        wv = w_in[2 * H + q0 : 2 * H + q0 + 256]
        in_maps.append(
            {
                "xT": xT[b],
                "wqkT": np.ascontiguousarray(np.concatenate([wq, wk], 0).T).astype(BF16NP),
                "wvT": np.ascontiguousarray(wv.T).astype(BF16NP),
                "bqkv": np.ascontiguousarray(
                    np.concatenate(
                        [b_in[q0 : q0 + 256], b_in[H + q0 : H + q0 + 256],
                         b_in[2 * H + q0 : 2 * H + q0 + 256]]
                    )
                ),
                "woT": np.ascontiguousarray(w_out[:, q0 : q0 + 256].T).astype(BF16NP),
                "maskd": mask,
                "identd": ident,
            }
        )
    return in_maps


def _ensure_ntff_hook():
    """Provide antenv.axon_hooks (NTFF profiling hook) if the container's
    antenv stub lacks it, by driving the axon .so C ABI directly. Also
    neuter the S3 artifact upload (zero-egress container)."""
    import contextlib
    import ctypes
    import types

    import concourse.bass_utils as bu

    bu.upload_artifacts = lambda tmpdir: str(tmpdir)
    try:
        from antenv.axon_hooks import get_axon_ntff_profile_hook  # noqa: F401

        return
    except ImportError:
        pass
    import antenv

    so_path = "/opt/axon/libaxon_pjrt.so"
    hook = None
    try:
        lib = ctypes.CDLL(so_path)
        if hasattr(lib, "axon_start_nrt_profile"):
            lib.axon_start_nrt_profile.argtypes = [
                ctypes.POINTER(ctypes.c_int64),
                ctypes.c_size_t,
            ]
            lib.axon_start_nrt_profile.restype = ctypes.c_int64
            lib.axon_stop_nrt_profile.argtypes = [ctypes.c_char_p]
            lib.axon_stop_nrt_profile.restype = ctypes.c_int64

            @contextlib.contextmanager
            def _hook(output_dir, device_ids):
                import jax

                jax.devices()
                if device_ids:
                    ids = (ctypes.c_int64 * len(device_ids))(*device_ids)
                    rc = lib.axon_start_nrt_profile(ids, len(device_ids))
                else:
                    rc = lib.axon_start_nrt_profile(None, 0)
                if rc != 0:
                    raise RuntimeError(f"axon_start_nrt_profile rc={rc}")
                try:
                    yield
                finally:
                    n = lib.axon_stop_nrt_profile(str(output_dir).encode())
                    print(f"ntff profile: {n} file(s) -> {output_dir}")

            hook = _hook
    except OSError:
        hook = None

    mod = types.ModuleType("antenv.axon_hooks")
    mod.get_axon_ntff_profile_hook = lambda: hook
    mod.set_axon_ntff_profile_hook = lambda h: None
    sys.modules["antenv.axon_hooks"] = mod
    antenv.axon_hooks = mod


def run_device(hidden_states, w_in, b_in, w_out, b_out, trace=False):
    """Returns (full output, BassKernelResults)."""
    if trace:
        _ensure_ntff_hook()
    nc = build_nc()
    in_maps = make_in_maps(hidden_states, w_in, b_in, w_out)
    res = run_bass_kernel_spmd(
        nc, in_maps, core_ids=list(range(8)), trace=trace
    )
    out = np.zeros((B, S, H), dtype=np.float32)
    for c in range(8):
        out[c // 4] += res.results[c]["outT"].T
    out += np.asarray(b_out, dtype=np.float32)[None, None, :]
    return out, res


def kernel(hidden_states, w_in, b_in, w_out, b_out):
    out, _ = run_device(hidden_states, w_in, b_in, w_out, b_out, trace=False)
    return out
